# revision 13
# baseline (speedup 1.0000x reference)
"""Trainium2 Bass kernel for nn_Attention_36361193128703 (self-contained).

Entry point: kernel(**inputs) -> np.ndarray
  inputs: x (2,2048,1024) f32, w_in (3072,1024) f32,
          kernel_offsets/amplitudes/sharpness (16,16) f32
  returns: (2, 2048, 1024) f32 attention output (matches reference).

Distribution: 8 NeuronCores = data-parallel over batch (2) x tensor-parallel
over heads (4 head-groups of 4). Each core runs an identical single-core Bass
program on its shard; outputs are concatenated on the host. No collectives.

Design notes (v2 rewrite):
 - TISA bias g = exp(bias) is a weight-only function; it is precomputed on
   the host into a per-head "staircase" table gb[p, u] = g[x0 + u + 127 - p]
   so the device multiplies P = exp(s) * g directly with plain slices.
 - g == 1 beyond a ~W-wide diagonal band (RBF kernels decay), so the DVE
   multiply is only emitted for tiles intersecting the band.
 - Attention main loop is paced by the ScalarE exp (the hard floor:
   16.8M exps/core @ 128/cycle @ 1.2 GHz). S matmuls write a 3-panel PSUM
   ring (6 banks); exp instructions cover 2 panels (N=2048) to amortize the
   ~352-cycle ACT instruction tail. AV accumulates in 1 bank; the last bank
   is a shared slot for projection accumulation and epilogue transposes.
 - S matmuls (contraction=64) for the even/odd head of a pair sit at
   base partitions 0/64 and are emitted adjacently so the PE runs them
   concurrently in separate row groups (2x).
 - Projection work is streamed through the attention loop as filler for the
   PE during ACT-paced windows.
"""
from contextlib import ExitStack

import numpy as np

import concourse.bass as bass
import concourse.mybir as mybir
import concourse.tile as tile
from concourse import bacc
from concourse.bass import AP

F32 = mybir.dt.float32
BF16 = mybir.dt.bfloat16

L = 2048
DM = 1024
HL = 4            # local heads (2 pairs)
HD = 64
JT = 128          # j (key) tile height
NJT = L // JT     # 16
IC = 256          # i (query) window per stage
NIC = L // IC     # 8 windows per pair
NDC = DM // 128   # 8 d-chunks for projection
WJT = 4           # j-tiles per exp window
NW = NJT // WJT   # 4 windows per stage
PAIR_S = False    # concurrent row-tiled S matmuls (corrupts on this walrus)
DEBUG_ES = False  # dump stage-(0,0) es tiles to a dbg output


def _band_params(w_used: int):
    """(UW, x0) for the host gb table; u0 = (i0 - j0) + w_used + 255."""
    uw = 2 * w_used + 640
    x0 = 1920 - w_used - 255
    return uw, x0


def build_kernel(w_used: int) -> bacc.Bacc:
    UW, _x0 = _band_params(w_used)
    nc = bacc.Bacc("TRN2", target_bir_lowering=False, debug=False, num_devices=8)

    xT_d = nc.dram_tensor("xT", [DM, L], BF16, kind="ExternalInput")
    wkqv_d = nc.dram_tensor("wkqv", [DM, 768], BF16, kind="ExternalInput")
    gb_d = nc.dram_tensor("gb", [HL * 128, UW], BF16, kind="ExternalInput")
    out_d = nc.dram_tensor("out", [L, 256], F32, kind="ExternalOutput")
    if DEBUG_ES:
        dbg_d = nc.dram_tensor("dbg", [4 * 128, 2048], BF16,
                               kind="ExternalOutput")

    dmae = [nc.sync, nc.gpsimd]

    def dma(i, out, in_):
        dmae[i % len(dmae)].dma_start(out, in_)

    def band_jts(i0):
        """Consecutive jt list whose tile [j0, j0+128) x [i0, i0+IC) meets
        |i - j| <= w_used."""
        lo = -(w_used + IC - 1)
        hi = w_used + 127
        out = []
        for jt in range(NJT):
            d = i0 - jt * JT
            if lo <= d <= hi:
                out.append(jt)
        return out

    with tile.TileContext(nc) as tc, ExitStack() as ctx:
        const_pool = ctx.enter_context(tc.tile_pool(name="const", bufs=1))
        xpool = ctx.enter_context(tc.tile_pool(name="xT", bufs=1))
        wpool = ctx.enter_context(tc.tile_pool(name="w", bufs=1))
        kqpool = ctx.enter_context(tc.tile_pool(name="kq", bufs=1))
        vpool = ctx.enter_context(tc.tile_pool(name="v", bufs=1))
        gpool = ctx.enter_context(tc.tile_pool(name="gb", bufs=1))
        espool = ctx.enter_context(tc.tile_pool(name="es", bufs=3))
        opool = ctx.enter_context(tc.tile_pool(name="o", bufs=2))
        otpool = ctx.enter_context(tc.tile_pool(name="ot", bufs=4))
        rcpool = ctx.enter_context(tc.tile_pool(name="rc", bufs=2))

        # PSUM: ring (6 banks) + AV accumulator (1 bank) + proj/aux (1 bank)
        ring_pool = ctx.enter_context(tc.tile_pool(name="ring", bufs=1, space="PSUM"))
        av_pool = ctx.enter_context(tc.tile_pool(name="avps", bufs=1, space="PSUM"))
        pj_pool = ctx.enter_context(tc.tile_pool(name="pjps", bufs=1, space="PSUM"))

        ring = ring_pool.tile([128, 3072], F32, tag="ring", name="ring")

        # ---- consts ----
        warm = const_pool.tile([128, 512], BF16, name="warm")
        nc.gpsimd.memset(warm[:, :], 0.125)
        from concourse.masks import make_identity
        identf = const_pool.tile([65, 65], F32, name="identf")
        make_identity(nc, identf[:, :])
        ident = const_pool.tile([65, 65], BF16, name="ident")
        nc.vector.tensor_copy(ident[:, :], identf[:, :])
        scr = const_pool.tile([128, 16], BF16, name="scr")

        # ---- HAM warm-up + ACT table preload (runs during input DMA) ----
        for i in range(10):
            nc.tensor.matmul(ring[:, 0:512], warm[:, 0:128], warm[:, :],
                             start=True, stop=True)
        nc.scalar.activation(scr[:, :], warm[:, 0:16],
                             mybir.ActivationFunctionType.Exp)

        # ---- input DMA ----
        w_sb = []
        for dc in range(NDC):
            wt = wpool.tile([128, 768], BF16, name=f"wk{dc}", tag=f"wk{dc}")
            dma(dc, wt[:, :], wkqv_d[dc * 128:(dc + 1) * 128, :])
            w_sb.append(wt)
        xT_sb = []
        for dc in range(NDC):
            xt = xpool.tile([128, L], BF16, name=f"xt{dc}", tag=f"xt{dc}")
            xT_sb.append(xt)
        for tcn in range(4):
            for dc in range(NDC):
                dma(tcn * NDC + dc, xT_sb[dc][:, tcn * 512:(tcn + 1) * 512],
                    xT_d[dc * 128:(dc + 1) * 128, tcn * 512:(tcn + 1) * 512])
        gb_sb = []
        for hg in range(HL):
            gt = gpool.tile([128, UW], BF16, name=f"gb{hg}", tag=f"gb{hg}")
            dma(hg, gt[:, :], gb_d[hg * 128:(hg + 1) * 128, :])
            gb_sb.append(gt)

        # ---- projection machinery ----
        # kq_sb[ec][tcn]: ec 0/1 = K pair0/1, 2/3 = Q pair0/1 (pre-scaled)
        kq_sb = [[None] * 4 for _ in range(4)]
        # kpad[p][h][tcn]: K for one head padded to 128 contraction rows with
        # zeros in the other head's 64 rows (used when PAIR_S is off)
        kpad = [[[None] * 4 for _ in range(2)] for _ in range(2)]
        v_sb = [None] * NJT

        def emit_kq_group(ec, tcn):
            pg = pj_pool.tile([128, 512], F32, tag="pj", name="pg")
            for k in range(NDC):
                dc = (tcn * 2 + k) % NDC
                nc.tensor.matmul(pg[:, :],
                                 w_sb[dc][:, ec * 128:(ec + 1) * 128],
                                 xT_sb[dc][:, tcn * 512:(tcn + 1) * 512],
                                 start=(k == 0), stop=(k == NDC - 1))
            if ec >= 2 or PAIR_S:
                kt = kqpool.tile([128, 512], BF16, name=f"kq{ec}_{tcn}",
                                 tag=f"kq{ec}_{tcn}")
                nc.vector.tensor_copy(kt[:, :], pg[:, :])
                kq_sb[ec][tcn] = kt
            else:
                for h in range(2):
                    kt = kqpool.tile([128, 512], BF16,
                                     name=f"kp{ec}_{h}_{tcn}",
                                     tag=f"kp{ec}_{h}_{tcn}")
                    nc.vector.memset(kt[64 - h * 64:128 - h * 64, :], 0.0)
                    nc.vector.tensor_copy(kt[h * 64:h * 64 + 64, :],
                                          pg[h * 64:h * 64 + 64, :])
                    kpad[ec][h][tcn] = kt

        def emit_v_group(tt):
            pg = pj_pool.tile([128, 512], F32, tag="pj", name="pg")
            for dc in range(NDC):
                nc.tensor.matmul(pg[:, 0:256],
                                 xT_sb[dc][:, tt * 128:(tt + 1) * 128],
                                 w_sb[dc][:, 512:768],
                                 start=(dc == 0), stop=(dc == NDC - 1))
            vt = vpool.tile([128, HL, 66], BF16, name=f"v{tt}", tag=f"v{tt}")
            nc.vector.memset(vt[:, :, 64:65], 1.0)
            for hg in range(HL):
                nc.vector.tensor_copy(vt[:, hg, 0:64],
                                      pg[:, hg * 64:(hg + 1) * 64])
            v_sb[tt] = vt

        # proj work queue: list of ('kq', ec, tcn) / ('v', tt)
        pq = []
        pq += [("v", tt) for tt in range(NJT)]
        pq += [("kq", 2, 1)]
        pq += [("kq", 1, t) for t in range(4)]
        pq += [("kq", 2, 2), ("kq", 2, 3)]
        pq += [("kq", 3, t) for t in range(4)]
        pq_pos = [0]

        def pump_proj(n):
            """Emit up to n whole proj groups from the queue."""
            for _ in range(n):
                if pq_pos[0] >= len(pq):
                    return
                item = pq[pq_pos[0]]
                pq_pos[0] += 1
                if item[0] == "kq":
                    emit_kq_group(item[1], item[2])
                else:
                    emit_v_group(item[1])

        def ensure_proj(pred):
            """Drain the queue until pred() says the prerequisite exists."""
            while not pred():
                assert pq_pos[0] < len(pq), "proj queue exhausted"
                pump_proj(1)

        # ---- prologue: K pair0 + Q pair0 (first token group) ----
        for tcn in range(4):
            emit_kq_group(0, tcn)
        emit_kq_group(2, 0)

        # ---- attention stages ----
        panel_ctr = [0]

        # panel layout (1024 f32 = 2 banks): [hA: jt_a | jt_a+1] [hB: jt_a | jt_a+1]
        # so the two concurrent row-tiled heads write different PSUM banks.
        def escol(k, h):
            """column in the es tile for jt-in-window k (0..3), head h."""
            return (k // 2) * 1024 + h * 512 + (k % 2) * 256

        def emit_S_window(p, i0, w):
            """S matmuls for window w (jts 4w..4w+3) -> 2 ring panels.
            Returns (offsets of the two panels)."""
            qt = kq_sb[2 + p][i0 // 512]
            qc = i0 % 512
            offs = []
            for half in range(2):
                poff = (panel_ctr[0] % 3) * 1024
                panel_ctr[0] += 1
                offs.append(poff)
                for js in range(2):
                    jt = 4 * w + 2 * half + js
                    j0 = jt * JT
                    jc = j0 % 512
                    for h in range(2):
                        c = poff + h * 512 + js * 256
                        if PAIR_S:
                            dA = h * 64
                            nc.tensor.matmul(
                                ring[:, c:c + 256],
                                kq_sb[p][j0 // 512][dA:dA + 64, jc:jc + JT],
                                qt[dA:dA + 64, qc:qc + 256],
                                start=True, stop=True)
                        else:
                            nc.tensor.matmul(
                                ring[:, c:c + 256],
                                kpad[p][h][j0 // 512][:, jc:jc + JT],
                                qt[:, qc:qc + 256],
                                start=True, stop=True)
            return offs

        def emit_exp(offs):
            """exp over the window's two panels -> es tile [128, 2048]."""
            es = espool.tile([128, 2048], BF16, tag="es", name="es")
            if offs[1] == offs[0] + 1024:
                nc.scalar.activation(es[:, :],
                                     ring[:, offs[0]:offs[0] + 2048],
                                     mybir.ActivationFunctionType.Exp)
            else:
                for half in range(2):
                    nc.scalar.activation(
                        es[:, half * 1024:half * 1024 + 1024],
                        ring[:, offs[half]:offs[half] + 1024],
                        mybir.ActivationFunctionType.Exp)
            return es

        def emit_band_muls(p, i0, w, es):
            bj = [jt for jt in band_jts(i0) if 4 * w <= jt < 4 * (w + 1)]
            for jt in bj:
                u0 = (i0 - jt * JT) + w_used + 255
                k = jt - 4 * w
                for h in range(2):
                    hg = 2 * p + h
                    sl = es[:, escol(k, h):escol(k, h) + 256]
                    nc.vector.tensor_mul(sl, sl, gb_sb[hg][:, u0:u0 + 256])

        def emit_AV_window(p, av_t, w, es):
            # av_t is a single PSUM bank shared by both heads: start=True
            # clears has_written for the WHOLE bank, so only the very first
            # matmul of the stage may set it (head B's first write then relies
            # on its bits being clear -> overwrite).
            for k in range(WJT):
                jt = 4 * w + k
                for h in range(2):
                    hg = 2 * p + h
                    nc.tensor.matmul(
                        av_t[:, h * 256:h * 256 + 256],
                        v_sb[jt][:, hg, 0:65],
                        es[:, escol(k, h):escol(k, h) + 256],
                        start=(jt == 0 and h == 0),
                        stop=(jt == NJT - 1 and h == 1),
                        skip_group_check=True)

        def emit_epilogue(p, i0, av_t):
            o_sb = opool.tile([65, 512], BF16, tag="o", name="o_sb")
            nc.vector.tensor_copy(o_sb[:, :], av_t[:, :])
            pt = pj_pool.tile([128, 4, 66], BF16, tag="pj", name="pt")
            for k in range(4):
                h, tq = k // 2, k % 2
                nc.tensor.transpose(pt[:, k, 0:65],
                                    o_sb[:, h * 256 + tq * 128:
                                         h * 256 + tq * 128 + 128],
                                    ident[:, :])
            rc = rcpool.tile([128, 4], F32, tag="rc", name="rc")
            nc.vector.reciprocal(rc[:, :], pt[:, :, 64])
            for k in range(4):
                h, tq = k // 2, k % 2
                hg = 2 * p + h
                ot = otpool.tile([128, HD], F32, tag="ot", name="ot")
                nc.vector.tensor_scalar(ot[:, :], pt[:, k, 0:64],
                                        rc[:, k:k + 1], None,
                                        op0=mybir.AluOpType.mult)
                nc.sync.dma_start(
                    out_d[i0 + tq * 128:i0 + (tq + 1) * 128,
                          hg * HD:(hg + 1) * HD],
                    ot[:, :])

        def k_ready(p):
            if PAIR_S:
                return all(kq_sb[p][t] is not None for t in range(4))
            return all(kpad[p][h][t] is not None
                       for h in range(2) for t in range(4))

        for p in range(2):
            # K tiles for this pair + Q for the first window must exist
            ensure_proj(lambda: k_ready(p))
            for icw in range(NIC):
                i0 = icw * IC
                ensure_proj(lambda: kq_sb[2 + p][i0 // 512] is not None)
                av_t = av_pool.tile([65, 512], F32, tag="av", name="av_t")
                prev = None
                for w in range(NW):
                    offs = emit_S_window(p, i0, w)
                    if prev is not None:
                        emit_AV_window(p, av_t, w - 1, prev)
                    es = emit_exp(offs)
                    emit_band_muls(p, i0, w, es)
                    if DEBUG_ES and p == 0 and icw == 0:
                        nc.sync.dma_start(
                            dbg_d[w * 128:(w + 1) * 128, :], es[:, :])
                    if p == 0 and icw == 0:
                        # V tiles are produced just-in-time in the first stage
                        ensure_proj(lambda: all(
                            v_sb[t] is not None for t in range(4 * w + 4)))
                    else:
                        pump_proj(1)
                    prev = es
                emit_AV_window(p, av_t, NW - 1, prev)
                emit_epilogue(p, i0, av_t)

        assert pq_pos[0] == len(pq)

    nc.compile()
    return nc


# ---------------------------------------------------------------------------
# host side
# ---------------------------------------------------------------------------

def _tisa_g(off, amp, sh):
    """g[h, 0:4095] = exp(bias scores) and the band half-width W."""
    rel = np.arange(-(L - 1), L, dtype=np.float64)
    diff = off.astype(np.float64)[:, :, None] - rel[None, None, :]
    sc = np.sum(amp.astype(np.float64)[:, :, None]
                * np.exp(-np.abs(sh.astype(np.float64))[:, :, None] * diff * diff),
                axis=1)                                   # (16, 4095)
    nz = np.abs(sc) > 1e-6
    w = 0
    for h in range(sc.shape[0]):
        idx = np.nonzero(nz[h])[0]
        if idx.size:
            w = max(w, int(np.max(np.abs(idx - (L - 1)))))
    g = np.exp(sc)
    return g, w


def _w_bucket(w):
    for b in (128, 256, 512, 1024, 1536):
        if w <= b:
            return b
    raise ValueError(f"band width {w} too large")


def shard_inputs(inputs: dict) -> list[dict]:
    """Full inputs -> 8 per-core input maps (bf16 prep for matmul operands)."""
    import ml_dtypes

    x, w_in = inputs["x"], inputs["w_in"]
    g, w = _tisa_g(inputs["kernel_offsets"], inputs["kernel_amplitudes"],
                   inputs["kernel_sharpness"])
    w_used = _w_bucket(max(w, 64))
    UW, x0 = _band_params(w_used)

    # gb[hg-local][p, u] = g[head, x0 + u + 127 - p], clamped to [0, 4094]
    p_idx = np.arange(128)
    u_idx = np.arange(UW)
    xi = x0 + u_idx[None, :] + 127 - p_idx[:, None]
    np.clip(xi, 0, 2 * L - 2 - 1, out=xi)  # valid g indices: [0, 4094]
    gb_by_head = [np.ascontiguousarray(g[h][xi]).astype(ml_dtypes.bfloat16)
                  for h in range(16)]

    D = DM
    in_maps = []
    xT_by_b = [np.ascontiguousarray(x[b].T).astype(ml_dtypes.bfloat16)
               for b in range(2)]
    wkqv_by_hg = []
    gb_by_hg = []
    for hg in range(4):
        heads = list(range(4 * hg, 4 * hg + 4))
        rows_k = np.concatenate([w_in[h * HD:(h + 1) * HD] for h in heads])
        rows_q = np.concatenate(
            [w_in[2 * D + h * HD:2 * D + (h + 1) * HD] for h in heads]
        ) * np.float32(1.0 / np.sqrt(HD))
        rows_v = np.concatenate([w_in[D + h * HD:(D + (h + 1) * HD)] for h in heads])
        wkqv = np.ascontiguousarray(
            np.concatenate([np.concatenate([rows_k, rows_q]).T, rows_v.T],
                           axis=1)).astype(ml_dtypes.bfloat16)
        wkqv_by_hg.append(wkqv)
        gb_by_hg.append(np.concatenate([gb_by_head[h] for h in heads], axis=0))
    for c in range(8):
        b, hg = c // 4, c % 4
        in_maps.append({"xT": xT_by_b[b], "wkqv": wkqv_by_hg[hg],
                        "gb": gb_by_hg[hg]})
    return in_maps


def unshard_output(results: list[dict]) -> np.ndarray:
    out = np.zeros((2, L, DM), np.float32)
    for c in range(8):
        b, hg = c // 4, c % 4
        out[b, :, hg * 256:(hg + 1) * 256] = results[c]["out"]
    return out


_NC_CACHE = None
_NC_W = None


def kernel(**inputs) -> np.ndarray:
    global _NC_CACHE, _NC_W
    from concourse.bass_utils import run_bass_kernel_spmd

    inputs = {k: np.asarray(v) for k, v in inputs.items()}
    _, w = _tisa_g(inputs["kernel_offsets"], inputs["kernel_amplitudes"],
                   inputs["kernel_sharpness"])
    w_used = _w_bucket(max(w, 64))
    if _NC_CACHE is None or _NC_W != w_used:
        _NC_CACHE = build_kernel(w_used)
        _NC_W = w_used
    in_maps = shard_inputs(inputs)
    res = run_bass_kernel_spmd(_NC_CACHE, in_maps, core_ids=list(range(8)))
    return unshard_output(res.results)
